# revision 35
# baseline (speedup 1.0000x reference)
# Gaussian-kernel ridge-regression matvec on 8 Trainium2 cores.
#
#   out_i = sum_j exp(-||x_i - y_j||^2 / g) * alpha_j
#   N=8192 queries, M=16384 train points, DIM=32, g scalar.
#
# Factorization (host prep is O(N+M), device does the O(N*M) part):
#   exp(-(x^2+y^2-2xy)/g)*a_j = exp(-x_i^2/g) * sign(a_j) * exp(s_ij),
#   s_ij = (2/g) x_i.y_j + c_j,   c_j = -y_j^2/g + ln|a_j|
# Train points are host-sorted so sign(a)>0 comes first (npos); row scale
# exp(-x_i^2/g) is applied on host.  s is computed by the fp16 hi/lo
# "triple"-product trick in ONE matmul pass (K = 3*33 = 99), pre-scaled by
# A = 2^23*log2(e) (split across the fp16 tables), so PSUM holds t = A*s.
# ScalarE's free affine (scale=1/A) undoes the prescale for exact exps.
#
# The all-ScalarE version is ACT-bound: FD=2048 exp from PSUM = ~1961ns,
# cadence ~2051ns x 64 groups.  This kernel breaks that floor two ways:
# 1. PE refill: the K=34 table (single-fp16 coords + hi/lo c rows) is
#    replicated at partitions 64..97, so each PSUM slot's four FD-512
#    matmuls issue as TWO CONCURRENT row-group pairs
#    (tile_position=(0,0)/(64,0), each pair writing separate banks),
#    cutting slot refill ~1865 -> ~1400ns.
# 2. Split-slot HYBRID groups: every non-boundary group's PSUM tile is
#    consumed by two engines IN PARALLEL (disjoint banks, both readers):
#    - ScalarE exps cols 0:1536 (FD=1536 ACTIVATE ~1538ns, fused
#      accum_out row-sum, +284ns accumulator read; the elementwise
#      output goes to a throwaway SBUF stage ON PURPOSE -- writing PSUM
#      in-place serializes against the DVE's concurrent read),
#    - VectorE Schraudolph-approximates cols 1536:2048 in ONE
#      tensor_scalar: i32 = convert(max(t, CLAMP) + MAGIC); the int32
#      bits bitcast to fp32 ARE 2^(t/2^23) = e^s to +-3% (mean bias
#      divided out on the host); its row-sum is a deferred bitcast
#      reduce.  Vector's in-order queue must carry ONLY these short ops:
#      any ~1.6us+ insertion delays the converts and inflates slot dwell.
# The sign-boundary group is a hybrid too, with exact sub-range reduces
# around the pos/neg split at r.  Cadence ~1910ns/group; HSPLIT=1536 is
# the measured optimum (1408 and staged-half variants both regress via
# Vector-queue delay).

import math
import numpy as np

N, M, DIM, NCORES = 8192, 16384, 32, 8
NLOC = N // NCORES
ITILES = NLOC // 128
GRP = 2048
NGRP = M // GRP
KAUG = DIM + 1
KPK = DIM + 2              # 34: single-fp16 coords + hi/lo c rows
KSTK = 64 + KPK            # 98: row-group copy at partitions 64..97
HSPLIT = 1536              # ACT columns per hybrid group (DVE gets GRP-HSPLIT)
HGRP = GRP // 2

# Schraudolph constants.  PSUM holds t = A*s with A = 2^23*log2(e).
A_SCALE = float(2 ** 23) / math.log(2.0)
AX = 2048.0                # xs coordinate scale (exact power of 2)
KAPPA = 32768.0            # c-row carrier (exact in fp16)
SCH_MAGIC = float(127 * 2 ** 23) - 366500.0
SCH_CLAMP = float(-119 * 2 ** 23)
SCH_BIAS = 0.00959         # mean relative error of the trick

HYB = 40                   # hybrid (ACT||DVE split-slot) groups per core
ACC_C = 0                  # ScalarE fused-accum full groups per core
DEFER = 1                  # groups to defer DVE row-sums by
OCOL = 18                  # parts/out columns per itile (9 base + extras)

_cache = {}


def _group_plan(npos):
    """Returns (hyb_set, acc_set) of (itile, group) units."""
    r = npos % GRP
    bg = npos // GRP if r else -1
    hyb, acc = set(), set()
    # per itile: boundary group exact, 2 staged groups (DVE TT+reduce sums),
    # everything else a uniform 1536/512 ACT/DVE hybrid
    for it in range(ITILES):
        for g in range(NGRP):
            if g != bg:
                hyb.add((it, g))
    return frozenset(hyb), frozenset(acc)


def _build(npos, hyb_set, acc_set):
    import concourse.bass as bass
    import concourse.tile as tile
    from concourse import bacc, mybir
    from collections import deque

    f32 = mybir.dt.float32
    f16 = mybir.dt.float16
    bf16 = mybir.dt.bfloat16
    i32 = mybir.dt.int32
    Exp = mybir.ActivationFunctionType.Exp
    X = mybir.AxisListType.X
    Add = mybir.AluOpType.add
    Max = mybir.AluOpType.max

    nc = bacc.Bacc("TRN2", target_bir_lowering=False, debug=False)
    ys = nc.dram_tensor("ys", [KSTK, M], f16, kind="ExternalInput").ap()
    xs = nc.dram_tensor("xs", [KSTK, NLOC], f16, kind="ExternalInput").ap()
    o = nc.dram_tensor("o", [128, ITILES * OCOL], f32,
                       kind="ExternalOutput").ap()

    r = npos % GRP
    bg = npos // GRP if r else -1
    gorder = list(range(NGRP))
    nsegcol = NGRP + (1 if bg >= 0 else 0)

    def gcols(g):  # group -> list of (sub0, sub1, is_pos)
        if g == bg:
            return [(0, r, True), (r, GRP, False)]
        return [(0, GRP, g * GRP < npos)]

    npc = sum(1 for g in range(NGRP) for s in gcols(g) if s[2])
    pcol = {}
    ip, ineg = 0, npc
    for g in range(NGRP):
        for (s0, s1, isp) in gcols(g):
            if isp:
                pcol[(g, s0)] = ip
                ip += 1
            else:
                pcol[(g, s0)] = ineg
                ineg += 1

    inv_a = 1.0 / A_SCALE

    with tile.TileContext(nc) as tc:
        with tc.tile_pool(name="ypool", bufs=1) as ypool, \
             tc.tile_pool(name="xpool", bufs=1) as xpool, \
             tc.tile_pool(name="psum", bufs=2, space="PSUM") as pp, \
             tc.tile_pool(name="stage", bufs=8) as stagep, \
             tc.tile_pool(name="istage", bufs=4) as istagep, \
             tc.tile_pool(name="parts", bufs=ITILES) as partp, \
             tc.tile_pool(name="res", bufs=1) as resp:

            # input DMAs on the two HWDGE queues (sync + scalar), issued in
            # the order the compute consumes them (proven baseline plan)
            ysts = []
            for ci in range(NGRP):
                t = ypool.tile([KSTK, GRP], f16, tag=f"ys{ci}")
                ysts.append(t)
            xst = xpool.tile([KSTK, NLOC], f16, tag="xs")
            nc.sync.dma_start(xst[:], xs[:])
            qs = [nc.scalar, nc.sync]
            for qi, g in enumerate(gorder):
                qs[qi % 2].dma_start(ysts[g][:], ys[:, g * GRP:(g + 1) * GRP])

            # PE warm-up: junk matmuls keep the PE primed while input DMAs
            # are in flight (proven baseline ramp behavior)
            warm = xpool.tile([KSTK, 512], f16, tag="warm")
            nc.gpsimd.memset(warm[:], 0.0)

            res = resp.tile([128, ITILES], f32)
            touched = set()
            deferred = deque()

            for it in range(ITILES):
                xw = xst[:, bass.ts(it, 128)]
                parts = partp.tile([128, OCOL], f32, tag="parts")
                xcol = nsegcol  # next free column for hybrid DVE halves

                for gi_pos, g in enumerate(gorder):
                    ps = pp.tile([128, GRP], f32, tag="ps")
                    # Pre-touch newly DMA'd tiles with dummy 1x1 matmuls into
                    # this PSUM tile (overwritten by the real matmuls below).
                    if it == 0:
                        if "xs" not in touched:
                            touched.add("xs")
                            for _ in range(12):
                                nc.tensor.matmul(ps[0:1, 0:512],
                                                 warm[:, 0:1], warm[:],
                                                 start=True, stop=True)
                            nc.tensor.matmul(ps[0:1, 1:2],
                                             xst[:, 0:1], xst[:, 0:1],
                                             start=True, stop=True)
                        nc.tensor.matmul(ps[0:1, 0:1],
                                         xst[:, 0:1], ysts[g][:, 0:1],
                                         start=True, stop=True)
                    for k in range(GRP // 512):
                        rlo = 0 if k % 2 == 0 else 64
                        nc.tensor.matmul(ps[:, bass.ts(k, 512)],
                                         xw[rlo:rlo + KPK, :],
                                         ysts[g][rlo:rlo + KPK,
                                                 bass.ts(k, 512)],
                                         start=True, stop=True,
                                         tile_position=(rlo, 0))

                    col = pcol[(g, 0)]
                    if g != bg and (it, g) in hyb_set:
                        # split-slot hybrid: ACT exps the low half (fused
                        # accum row-sum to the base column) while the DVE
                        # Schraudolph-converts the high half; both only
                        # READ disjoint bank ranges of ps, so they overlap.
                        st = stagep.tile([128, HSPLIT], bf16, tag="st")
                        nc.scalar.activation(st[:], ps[:, 0:HSPLIT], Exp,
                                             scale=inv_a,
                                             accum_out=parts[:, col:col + 1])
                        ist = istagep.tile([128, GRP - HSPLIT], i32, tag="ist")
                        nc.vector.tensor_scalar(ist[:], ps[:, HSPLIT:GRP],
                                                SCH_CLAMP, SCH_MAGIC,
                                                Max, Add)

                        def sums_h(ist=ist, parts=parts, xc=xcol):
                            nc.vector.reduce_sum(parts[:, xc:xc + 1],
                                                 ist[:].bitcast(f32), axis=X)
                        deferred.append(sums_h)
                        xcol += 1
                    elif g != bg and (it, g) in acc_set:
                        # exp with fused ScalarE accumulator (no DVE work)
                        nc.scalar.activation(ps[:], ps[:], Exp, scale=inv_a,
                                             accum_out=parts[:, col:col + 1])
                    elif g == bg:
                        # sign-boundary group, hybridized: ScalarE exps the
                        # low HSPLIT cols (staged or accum'd depending on
                        # where the pos/neg boundary r falls), the DVE
                        # Schraudolphs the high cols; sub-range reduces keep
                        # the split exact.
                        ist = istagep.tile([128, GRP - HSPLIT], i32,
                                           tag="istb")
                        nc.vector.tensor_scalar(ist[:], ps[:, HSPLIT:GRP],
                                                SCH_CLAMP, SCH_MAGIC,
                                                Max, Add)
                        if r <= HSPLIT:
                            st = stagep.tile([128, HSPLIT], bf16, tag="stb")
                            nc.scalar.activation(st[:], ps[:, 0:HSPLIT], Exp,
                                                 scale=inv_a)

                            def sums_bg(st=st, ist=ist, parts=parts,
                                        xc=xcol):
                                colp = pcol[(bg, 0)]
                                coln = pcol[(bg, r)]
                                nc.vector.reduce_sum(
                                    parts[:, colp:colp + 1], st[:, 0:r],
                                    axis=X)
                                if r < HSPLIT:
                                    nc.vector.reduce_sum(
                                        parts[:, coln:coln + 1],
                                        st[:, r:HSPLIT], axis=X)
                                # high cols are all neg here (r <= HSPLIT)
                                nc.vector.reduce_sum(
                                    parts[:, xc:xc + 1],
                                    ist[:].bitcast(f32), axis=X)
                        else:
                            # pos/neg boundary falls in the DVE segment:
                            # ACT half is all-pos -> fused accum
                            nc.scalar.activation(
                                ps[:, 0:HSPLIT], ps[:, 0:HSPLIT], Exp,
                                scale=inv_a,
                                accum_out=parts[:, pcol[(bg, 0)]:
                                                pcol[(bg, 0)] + 1])

                            def sums_bg(ist=ist, parts=parts, xc=xcol):
                                fv = ist[:].bitcast(f32)
                                coln = pcol[(bg, r)]
                                nc.vector.reduce_sum(
                                    parts[:, xc:xc + 1],
                                    fv[:, 0:r - HSPLIT], axis=X)
                                nc.vector.reduce_sum(
                                    parts[:, coln:coln + 1],
                                    fv[:, r - HSPLIT:GRP - HSPLIT], axis=X)
                        deferred.append(sums_bg)
                        xcol += 1
                    else:
                        st = stagep.tile([128, GRP], bf16, tag="stb")
                        nc.scalar.activation(st[:], ps[:], Exp, scale=inv_a)

                        def sums_st(st=st, parts=parts, col=col):
                            h = stagep.tile([128, HGRP], bf16, tag="h")
                            nc.vector.tensor_add(h[:], st[:, 0:HGRP],
                                                 st[:, HGRP:GRP])
                            nc.vector.reduce_sum(parts[:, col:col + 1],
                                                 h[:], axis=X)
                        deferred.append(sums_st)
                    while len(deferred) > DEFER:
                        deferred.popleft()()

                while deferred:
                    deferred.popleft()()
                # stream the raw per-group sums out; the pos/neg combine
                # is O(N) and runs on the host
                nc.sync.dma_start(o[:, it * OCOL:it * OCOL + xcol], parts[:, 0:xcol])

    nc.compile()
    return nc


def kernel(x, y_train, alphas, g):
    from concourse.bass_utils import run_bass_kernel_spmd

    x = np.asarray(x, dtype=np.float32)
    y_train = np.asarray(y_train, dtype=np.float32)
    a = np.asarray(alphas, dtype=np.float32).reshape(-1)
    gf = float(np.asarray(g).reshape(-1)[0])

    y2 = np.sum(y_train.astype(np.float64) ** 2, axis=1)
    with np.errstate(divide="ignore"):
        c = -y2 / gf + np.log(np.abs(a.astype(np.float64)))
    c = np.maximum(c, -120.0)

    pos = a >= 0
    order = np.concatenate([np.nonzero(pos)[0], np.nonzero(~pos)[0]])
    npos = int(pos.sum())

    # Scale split: xs rows carry x*AX (+ kappa in the augmented row), ys rows
    # carry y*(2/g)*AY (+ c*AC), with AX*AY = kappa*AC = A = 2^23*log2(e),
    # so PSUM = A*s exactly (in fp16 hi/lo triple precision).
    AY = A_SCALE / AX
    AC = A_SCALE / KAPPA

    ytab = np.zeros((KSTK, M), dtype=np.float16)
    yc = (2.0 / gf) * AY * y_train[order].T.astype(np.float64)
    cs = AC * c[order]
    ch = cs.astype(np.float16)
    cl = (cs - ch.astype(np.float64)).astype(np.float16)
    ytab[:DIM] = yc.astype(np.float16)
    ytab[DIM] = ch
    ytab[DIM + 1] = cl
    ytab[64:64 + KPK] = ytab[0:KPK]
    ysn = ytab  # [98, M]

    hyb_set, acc_set = _group_plan(npos)
    key = npos
    if key not in _cache:
        _cache[key] = _build(npos, hyb_set, acc_set)
    nc = _cache[key]

    in_maps = []
    for k in range(NCORES):
        xsl = x[k * NLOC:(k + 1) * NLOC]
        xtab = np.zeros((KSTK, NLOC), dtype=np.float16)
        xtab[:DIM] = (AX * xsl.T.astype(np.float64)).astype(np.float16)
        xtab[DIM] = KAPPA
        xtab[DIM + 1] = KAPPA
        xtab[64:64 + KPK] = xtab[0:KPK]
        in_maps.append({"ys": ysn, "xs": xtab})  # [98, NLOC]

    r = run_bass_kernel_spmd(nc, in_maps, core_ids=list(range(NCORES)))

    r_ = npos % M % GRP
    bg = npos // GRP if r_ else -1
    nsegcol = NGRP + (1 if bg >= 0 else 0)
    npc = (bg + 1) if bg >= 0 else sum(
        1 for gg in range(NGRP) if gg * GRP < npos)

    pcol = {}
    ip, ineg = 0, npc
    for gg in range(NGRP):
        segs = [(0, r_, True), (r_, GRP, False)] if gg == bg else \
               [(0, GRP, gg * GRP < npos)]
        for (s0, s1, isp) in segs:
            if isp:
                pcol[(gg, s0)] = ip
                ip += 1
            else:
                pcol[(gg, s0)] = ineg
                ineg += 1

    # sign and Schraudolph-bias bookkeeping for the hybrid DVE-half columns:
    # column assignment follows _build's emission order (gorder order)
    HS = HSPLIT
    hyb_cols = []  # (it, xcol, sign)
    for it in range(ITILES):
        xcol = nsegcol
        for gg in range(NGRP):
            if gg != bg and (it, gg) in hyb_set:
                hyb_cols.append((it, xcol, 1.0 if gg * GRP < npos else -1.0))
                xcol += 1
            elif gg == bg:
                # boundary group's DVE segment: all-neg if r_ <= HS, else
                # the pos remainder [HS:r_)
                hyb_cols.append((it, xcol, -1.0 if r_ <= HS else 1.0))
                xcol += 1

    x2 = np.sum(x.astype(np.float64) ** 2, axis=1)
    rowscale = np.exp(-x2 / gf)
    out = np.empty(N, dtype=np.float64)
    corr = 1.0 / (1.0 + SCH_BIAS)
    for k in range(NCORES):
        parts = r.results[k]["o"].reshape(128, ITILES, OCOL).astype(np.float64)
        base = parts[:, :, :nsegcol]
        res = (base[:, :, :npc].sum(axis=2)
               - base[:, :, npc:].sum(axis=2))  # [128, ITILES]
        for (it, xc, sgn) in hyb_cols:
            res[:, it] += sgn * corr * parts[:, it, xc]
        out[k * NLOC:(k + 1) * NLOC] = res.T.reshape(NLOC)
    out *= rowscale
    return out.astype(np.float32).reshape(N, 1)
